# revision 34
# baseline (speedup 1.0000x reference)
"""Multi-head cross-attention (B=32, N=128, M=2048, 16 heads x 64) on 8 TRN2 cores.

Strategy: pure data-parallel over the batch dim (4 batches/core). All matmul
operands are fp16 (fp32 accumulation in PSUM); softmax skips the max-subtraction
(scores are ~N(0,1), |s|max ~ 6.5, exp stays well inside fp16 range) and the
row-sum is fused into the AV matmul as a 65th ones-column of V. Inputs are
transposed/cast on the host so every device-side matmul contracts over the
partition dim with no on-chip transposes.

Per-core device program (b = 4 batches):
  qhT  = Wq^T q^T            (heads on partitions, scale 1/8 folded into q)
  per batch:
    vh   = kv Wv             (kv tokens on partitions; per head a 65-col block:
                              even heads [d|1], odd heads [1|d])
    khT_c = Wk^T kv^T        (per inner chunk c of 128 = head pair)
    scoresT = khT_c^T qhT    (K=128 with the other head's q half zeroed)
    attnT = exp(scoresT)     (ACT, fp16)
    avT   = [vh | 1]^T attnT (65 x 128 per head; row 64 = softmax denominator)
    outT_h = avT_h * (1/denom)  (scalar cross-partition copy of the denom row
                                 -> recip_approx_fast -> gpsimd partition
                                 broadcast -> DVE mul writing even heads to
                                 outt2[0:64] and odd heads to outt2[64:128]
                                 via an offset output AP; no PE involvement)
    y = outT^T Wo + bo       (K=128 per head pair, accumulated over 8 chunks)

DMA triggers cost ~0.8us each on the sync engine, so inputs are batched into
few transfers ordered by first use (qT/Wq k-pairs feeding a k-outer qproj,
then kv[0]/Wv halves matching the n2-outer vproj, then Wk/Wo/bo)."""
import numpy as np

NCORES = 8
B, BPC = 32, 4
N, M = 128, 2048
H, D = 16, 64
QD, KVD, INNER = 1024, 512, 1024

_cached = {}


def _build_nc():
    from contextlib import ExitStack

    import concourse.tile as tile
    from concourse import bacc, mybir

    F16 = mybir.dt.float16
    F32 = mybir.dt.float32
    AF = mybir.ActivationFunctionType

    nc = bacc.Bacc("TRN2", target_bir_lowering=False, debug=False,
                   num_devices=NCORES)
    qT_d = nc.dram_tensor("qT", [QD, BPC * N], F16, kind="ExternalInput").ap()
    kvT_d = nc.dram_tensor("kvT", [BPC, KVD, M], F16, kind="ExternalInput").ap()
    wq_d = nc.dram_tensor("Wq", [QD, INNER], F16, kind="ExternalInput").ap()
    wk_d = nc.dram_tensor("Wk", [KVD, INNER], F16, kind="ExternalInput").ap()
    wv_d = nc.dram_tensor("Wv", [KVD, INNER], F16, kind="ExternalInput").ap()
    wo_d = nc.dram_tensor("Wo", [INNER, QD], F16, kind="ExternalInput").ap()
    bo_d = nc.dram_tensor("bo", [128, QD], F16, kind="ExternalInput").ap()
    y_d = nc.dram_tensor("y", [BPC, N, QD], F16, kind="ExternalOutput").ap()

    with tile.TileContext(nc) as tc, ExitStack() as ctx:
        const = ctx.enter_context(tc.tile_pool(name="const", bufs=1))
        kvt_pool = ctx.enter_context(tc.tile_pool(name="kvt", bufs=2))
        kht_pool = ctx.enter_context(tc.tile_pool(name="kht", bufs=3))
        vh_pool = ctx.enter_context(tc.tile_pool(name="vh", bufs=1))
        attn_pool = ctx.enter_context(tc.tile_pool(name="attn", bufs=20))
        outt_pool = ctx.enter_context(tc.tile_pool(name="outt", bufs=2))
        y_pool = ctx.enter_context(tc.tile_pool(name="yp", bufs=2))
        r_pool = ctx.enter_context(tc.tile_pool(name="rp", bufs=8))
        rb_pool = ctx.enter_context(tc.tile_pool(name="rb", bufs=2))
        pp = ctx.enter_context(tc.tile_pool(name="pp", bufs=3, space="PSUM"))
        scp = ctx.enter_context(tc.tile_pool(name="scp", bufs=3, space="PSUM"))
        avp = ctx.enter_context(tc.tile_pool(name="avp", bufs=2, space="PSUM"))

        BN = BPC * N  # 512

        # ---- inputs. DMA flows in issue order at full bandwidth after a
        # fixed ~9us engine-startup latency, so the k-interleaved qT/Wq
        # chunks (384KB per k-step) let the k-outer q-projection start ~1us
        # after first bytes instead of waiting for the full 3MB. kv[0]+Wv
        # next (vproj b0), then Wk/Wo/bo which are needed later.
        # Each dma_start costs the sync engine ~0.8us of serial trigger time,
        # so batch into k-PAIRS: 8 triggers for qT+Wq instead of 16, with the
        # k=0,1 chunks (375KB) arriving ~10us in for the k-outer qproj start.
        qt_sb = const.tile([128, 8 * BN], F16)
        qt_v = qt_sb[:].rearrange("p (k n) -> p k n", k=8)
        qt_dv = qT_d.rearrange("(k p) n -> p k n", p=128)
        wq_sb = const.tile([128, 8 * INNER], F16)
        wq_v = wq_sb[:].rearrange("p (k n) -> p k n", k=8)
        wq_dv = wq_d.rearrange("(k p) n -> p k n", p=128)
        # k=0 and k=1 as singles (first matmul waits only 375KB/2), rest as
        # pairs to bound the per-dma trigger tax.
        for ks, ke in ((0, 1), (1, 2), (2, 4), (4, 6), (6, 8)):
            nc.sync.dma_start(qt_v[:, ks:ke, :], qt_dv[:, ks:ke, :])
            nc.sync.dma_start(wq_v[:, ks:ke, :], wq_dv[:, ks:ke, :])

        kvt_tiles = {}

        def load_kvt(bb, split=False):
            t = kvt_pool.tile([128, 4 * M], F16, tag="kvt")
            tv = t[:].rearrange("p (k h m) -> p k h m", k=4, h=2)
            dv = kvT_d[bb].rearrange("(k p) (h m) -> p k h m", p=128, h=2)
            if split:
                return t, tv, dv
            nc.sync.dma_start(tv[:], dv[:])
            kvt_tiles[bb] = t

        # b0's kv and Wv stream as column/n2 halves interleaved so vproj's
        # first matmuls (t<8, n2=0) start one half-arrival earlier.
        kvt0, kvt0_v, kvt0_dv = load_kvt(0, split=True)
        kvt_tiles[0] = kvt0
        wv_sb = const.tile([128, 4 * INNER], F16)
        wv_v = wv_sb[:].rearrange("p (k h n) -> p k h n", k=4, h=2)
        wv_dv = wv_d.rearrange("(k p) (h n) -> p k h n", p=128, h=2)
        nc.sync.dma_start(kvt0_v[:, :, 0, :], kvt0_dv[:, :, 0, :])
        nc.sync.dma_start(wv_v[:, :, 0, :], wv_dv[:, :, 0, :])
        nc.sync.dma_start(kvt0_v[:, :, 1, :], kvt0_dv[:, :, 1, :])
        nc.sync.dma_start(wv_v[:, :, 1, :], wv_dv[:, :, 1, :])
        wk_sb = const.tile([128, 4 * INNER], F16)
        nc.sync.dma_start(
            wk_sb[:].rearrange("p (k n) -> p k n", k=4),
            wk_d.rearrange("(k p) n -> p k n", p=128),
        )
        wo_sb = const.tile([128, 8 * QD], F16)
        nc.sync.dma_start(
            wo_sb[:].rearrange("p (k n) -> p k n", k=8),
            wo_d.rearrange("(k p) n -> p k n", p=128),
        )
        bo_bc = const.tile([128, QD], F16)
        nc.sync.dma_start(bo_bc[:], bo_d[:])

        # ---- qhT projection: all 4 batches at once, chunk c = head pair.
        # Layout (c, b, hi, n): per (c, b) the two head-halves sit in adjacent
        # 128-col blocks, each with the complementary 64 partition rows zeroed,
        # so ONE K=128 N=256 scores matmul serves both heads of the pair.
        qh2 = const.tile([128, 8 * 2 * BN], F16)
        qh2v = qh2[:].rearrange("p (c b i n) -> p c b i n", c=8, b=BPC, i=2)
        nc.vector.memset(qh2v[64:128, :, :, 0, :], 0.0)
        nc.vector.memset(qh2v[0:64, :, :, 1, :], 0.0)
        # k-outer accumulation across all 8 PSUM banks (borrowed from the
        # pp/scp/avp rings, unused this early) so the first matmul only waits
        # for the k=0 qT/Wq chunks.
        qps = []
        for c in range(8):
            pool_c = pp if c < 3 else (scp if c < 6 else avp)
            qp = pool_c.tile([128, BN], F32, tag=("pp", "scp", "avp")[min(c // 3, 2)],
                             name=f"qp{c}")
            qps.append(qp)
        for k in range(8):
            for c in range(8):
                nc.tensor.matmul(
                    qps[c][:],
                    wq_sb[:, k * INNER + c * 128:k * INNER + (c + 1) * 128],
                    qt_sb[:, k * BN:(k + 1) * BN],
                    start=(k == 0), stop=(k == 7),
                )
        # evictions on Scalar (idle until the first EXP) so the Vector queue
        # is free for vproj's PSUM->vh casts right after.
        for c in range(8):
            nc.scalar.activation(
                qh2v[0:64, c, :, 0, :],
                qps[c][0:64, :].rearrange("p (b n) -> p b n", b=BPC),
                AF.Copy,
            )
            nc.scalar.activation(
                qh2v[64:128, c, :, 1, :],
                qps[c][64:128, :].rearrange("p (b n) -> p b n", b=BPC),
                AF.Copy,
            )

        vh_sb = vh_pool.tile([128, 16 * H * 65], F16, tag="vh")
        vh4 = vh_sb[:].rearrange("p (t h d) -> p t h d", h=H, d=65)
        nc.vector.memset(vh4[:, :, :, 64:65], 1.0)

        # Out-projection for a finished batch: fold odd heads into rows
        # 64-127 (one rectangular partition-shift DMA), K=128 accumulate
        # over the 8 inner chunks, add bias, store fp16 halves. Deferred
        # until after the NEXT batch's vh matmuls so its serialized tail
        # hides under PE work.
        def do_outproj(work):
            bb, o2 = work
            y_sb = y_pool.tile([128, QD], F16, tag="yp")
            yp0 = pp.tile([128, 512], F32, tag="pp")
            yp1 = pp.tile([128, 512], F32, tag="pp")
            yps = [yp0, yp1]
            def add_store(n2):
                nc.vector.tensor_add(
                    y_sb[:, n2 * 512:(n2 + 1) * 512],
                    yps[n2][:],
                    bo_bc[:, n2 * 512:(n2 + 1) * 512],
                )
                nc.sync.dma_start(y_d[bb, :, n2 * 512:(n2 + 1) * 512],
                                  y_sb[:, n2 * 512:(n2 + 1) * 512])

            # n2-outer: the n2=0 add+store hides under the n2=1 group's
            # eight matmuls instead of just the final one.
            for n2 in range(2):
                for c3 in range(8):
                    nc.tensor.matmul(
                        yps[n2][:],
                        o2[:, c3 * N:(c3 + 1) * N],
                        wo_sb[:, c3 * QD + n2 * 512:c3 * QD + (n2 + 1) * 512],
                        start=(c3 == 0), stop=(c3 == 7),
                    )
                add_store(n2)

        pending_proj = None
        for b in range(BPC):
            if b + 1 < BPC:
                load_kvt(b + 1)
            kvt_sb = kvt_tiles.pop(b)

            # ---- vh = kv @ Wv, kv tokens on partitions, 65-col blocks per
            # head. n2-outer so the first pass consumes only the first
            # Wv/kv column halves, matching the DMA arrival order for b0.
            for n2 in range(2):
                for t in range(16):
                    p = pp.tile([128, 512], F32, tag="pp")
                    for k in range(4):
                        nc.tensor.matmul(
                            p[:],
                            kvt_sb[:, k * M + t * 128:k * M + (t + 1) * 128],
                            wv_sb[:, k * INNER + n2 * 512:k * INNER + (n2 + 1) * 512],
                            start=(k == 0), stop=(k == 3),
                        )
                    nc.vector.tensor_copy(
                        vh4[:, t, n2 * 8:(n2 + 1) * 8, 0:64],
                        p[:].rearrange("p (h d) -> p h d", d=64),
                    )

            if pending_proj is not None:
                do_outproj(pending_proj)
                pending_proj = None

            # ---- per head pair: khT chunk -> scores -> exp; AV runs one
            # pair behind so the ACT exp latency hides under PE work.
            pending = None

            def do_av(avwork):
                at_list, cc = avwork
                p = avp.tile([128, 256], F32, tag="avp")
                for hi in range(2):
                    h = 2 * cc + hi
                    for t in range(16):
                        tg, j = t // 2, t % 2
                        nc.tensor.matmul(
                            p[0:65, hi * 128:(hi + 1) * 128],
                            vh4[:, t, h, :],
                            at_list[tg][:, j * 256 + hi * 128:j * 256 + (hi + 1) * 128],
                            start=(t == 0), stop=(t == 15),
                        )
                # normalize: denom (row 64) -> partition 0 (scalar ACT copy;
                # custom-DVE ops only work at partition base 0) -> 1/denom ->
                # broadcast down partitions -> multiply. No PE involvement.
                rb = rb_pool.tile([64, 2 * N], F32, tag="rb")
                dce = r_pool.tile([1, N], F32, tag="ce")
                dco = r_pool.tile([1, N], F32, tag="co")
                r32e = r_pool.tile([1, N], F32, tag="re")
                r32o = r_pool.tile([1, N], F32, tag="ro")
                nc.scalar.activation(dce[:], p[64:65, 0:128], AF.Copy)
                nc.scalar.activation(dco[:], p[64:65, 128:256], AF.Copy)
                nc.vector.reciprocal_approx_fast(r32e[:], dce[:])
                nc.vector.reciprocal_approx_fast(r32o[:], dco[:])
                nc.gpsimd.partition_broadcast(rb[:, 0:N], r32e[:])
                nc.gpsimd.partition_broadcast(rb[:, N:2 * N], r32o[:])
                # DVE output APs may sit at a different partition base than
                # the (base-aligned) inputs, so the odd head lands directly
                # in rows 64:128 -- no partition-fold DMA needed.
                nc.vector.tensor_mul(outt2[0:64, cc * N:(cc + 1) * N],
                                     p[0:64, 0:128], rb[:, 0:N])
                nc.vector.tensor_mul(outt2[64:128, cc * N:(cc + 1) * N],
                                     p[0:64, 128:256], rb[:, N:2 * N])

            outt2 = outt_pool.tile([128, 8 * N], F16, tag="outt2")
            for c in range(8):
                kht_sb = kht_pool.tile([128, M], F16, tag="kht")
                for n in range(4):
                    p = pp.tile([128, 512], F32, tag="pp")
                    for k in range(4):
                        nc.tensor.matmul(
                            p[:],
                            wk_sb[:, k * INNER + c * 128:k * INNER + (c + 1) * 128],
                            kvt_sb[:, k * M + n * 512:k * M + (n + 1) * 512],
                            start=(k == 0), stop=(k == 3),
                        )
                    nc.vector.tensor_copy(kht_sb[:, n * 512:(n + 1) * 512], p[:])

                at_tiles = []
                for tg in range(8):
                    sc = scp.tile([128, 512], F32, tag="scp")
                    for j in range(2):
                        t = tg * 2 + j
                        nc.tensor.matmul(
                            sc[:, j * 256:(j + 1) * 256],
                            kht_sb[:, t * 128:(t + 1) * 128],
                            qh2[:, (c * BPC + b) * 256:(c * BPC + b + 1) * 256],
                            start=True, stop=True,
                        )
                    at = attn_pool.tile([128, 512], F16, tag="attn")
                    nc.scalar.activation(at[:], sc[:], AF.Exp)
                    at_tiles.append(at)

                if pending is not None:
                    do_av(pending)
                pending = (at_tiles, c)
            do_av(pending)

            pending_proj = (b, outt2)
        do_outproj(pending_proj)

    nc.compile()
    return nc


def _get_nc():
    if "nc" not in _cached:
        _cached["nc"] = _build_nc()
    return _cached["nc"]


def kernel(q, kv, Wq, Wk, Wv, Wo, bo):
    from concourse.bass_utils import run_bass_kernel_spmd

    nc = _get_nc()

    wq16 = Wq.astype(np.float16)
    wk16 = Wk.astype(np.float16)
    wv16 = Wv.astype(np.float16)
    wo16 = Wo.astype(np.float16)
    bo16 = np.ascontiguousarray(
        np.broadcast_to(bo.reshape(1, QD), (128, QD)).astype(np.float16))

    scale = D ** -0.5  # 1/8, exact in fp16
    in_maps = []
    for i in range(NCORES):
        bs = slice(i * BPC, (i + 1) * BPC)
        # (BPC, N, QD) -> (QD, BPC, N) -> (QD, BPC*N), scale folded in
        qT = np.ascontiguousarray(
            np.transpose(q[bs] * scale, (2, 0, 1)).reshape(QD, BPC * N)
        ).astype(np.float16)
        kvT = np.ascontiguousarray(np.transpose(kv[bs], (0, 2, 1))).astype(
            np.float16
        )
        in_maps.append(
            {"qT": qT, "kvT": kvT, "Wq": wq16, "Wk": wk16, "Wv": wv16,
             "Wo": wo16, "bo": bo16}
        )

    _cached["in_maps"] = in_maps
    res = run_bass_kernel_spmd(nc, in_maps, list(range(NCORES)))
    out = np.concatenate([res.results[i]["y"] for i in range(NCORES)], axis=0)
    return out.astype(np.float32)


# revision 39
# speedup vs baseline: 1.0694x; 1.0694x over previous
"""Multi-head cross-attention (B=32, N=128, M=2048, 16 heads x 64) on 8 TRN2 cores.

Strategy: pure data-parallel over the batch dim (4 batches/core). All matmul
operands are fp16 (fp32 accumulation in PSUM); softmax skips the max-subtraction
(scores are ~N(0,1), |s|max ~ 6.5, exp stays well inside fp16 range) and the
row-sum is fused into the AV matmul as a 65th ones-column of V. Inputs are
transposed/cast on the host so every device-side matmul contracts over the
partition dim with no on-chip transposes.

Per-core device program (b = 4 batches):
  qhT  = Wq^T q^T            (heads on partitions, scale 1/8 folded into q)
  per batch:
    vh   = kv Wv             (kv tokens on partitions; per head a 65-col block:
                              even heads [d|1], odd heads [1|d])
    khT_c = Wk^T kv^T        (per inner chunk c of 128 = head pair)
    scoresT = khT_c^T qhT    (K=128 with the other head's q half zeroed)
    attnT = exp(scoresT)     (ACT, fp16)
    avT   = [vh | 1]^T attnT (65 x 128 per head; row 64 = softmax denominator)
    outT_h = avT_h * (1/denom)  (scalar cross-partition copy of the denom row
                                 -> recip_approx_fast -> gpsimd partition
                                 broadcast -> DVE mul writing even heads to
                                 outt2[0:64] and odd heads to outt2[64:128]
                                 via an offset output AP; no PE involvement)
    y = outT^T Wo + bo       (K=128 per head pair, accumulated over 8 chunks)

DMA triggers cost ~0.8us each on the sync engine, so inputs are batched into
few transfers ordered by first use (qT/Wq k-pairs feeding a k-outer qproj,
then kv[0]/Wv halves matching the n2-outer vproj, then Wk/Wo/bo)."""
import numpy as np

NCORES = 8
B, BPC = 32, 4
N, M = 128, 2048
H, D = 16, 64
QD, KVD, INNER = 1024, 512, 1024

_cached = {}


def _build_nc():
    from contextlib import ExitStack

    import concourse.tile as tile
    from concourse import bacc, mybir

    F16 = mybir.dt.float16
    F32 = mybir.dt.float32
    AF = mybir.ActivationFunctionType

    nc = bacc.Bacc("TRN2", target_bir_lowering=False, debug=False,
                   num_devices=NCORES)
    qT_d = nc.dram_tensor("qT", [QD, BPC * N], F16, kind="ExternalInput").ap()
    kvT_d = nc.dram_tensor("kvT", [BPC, KVD, M], F16, kind="ExternalInput").ap()
    wq_d = nc.dram_tensor("Wq", [QD, INNER], F16, kind="ExternalInput").ap()
    wk_d = nc.dram_tensor("Wk", [KVD, INNER], F16, kind="ExternalInput").ap()
    wv_d = nc.dram_tensor("Wv", [KVD, INNER], F16, kind="ExternalInput").ap()
    wo_d = nc.dram_tensor("Wo", [INNER, QD], F16, kind="ExternalInput").ap()
    bo_d = nc.dram_tensor("bo", [128, QD], F16, kind="ExternalInput").ap()
    y_d = nc.dram_tensor("y", [BPC, N, QD], F16, kind="ExternalOutput").ap()

    with tile.TileContext(nc) as tc, ExitStack() as ctx:
        const = ctx.enter_context(tc.tile_pool(name="const", bufs=1))
        kvt_pool = ctx.enter_context(tc.tile_pool(name="kvt", bufs=2))
        kht_pool = ctx.enter_context(tc.tile_pool(name="kht", bufs=3))
        vh_pool = ctx.enter_context(tc.tile_pool(name="vh", bufs=1))
        attn_pool = ctx.enter_context(tc.tile_pool(name="attn", bufs=20))
        outt_pool = ctx.enter_context(tc.tile_pool(name="outt", bufs=2))
        y_pool = ctx.enter_context(tc.tile_pool(name="yp", bufs=2))
        r_pool = ctx.enter_context(tc.tile_pool(name="rp", bufs=8))
        rb_pool = ctx.enter_context(tc.tile_pool(name="rb", bufs=2))
        pp = ctx.enter_context(tc.tile_pool(name="pp", bufs=3, space="PSUM"))
        scp = ctx.enter_context(tc.tile_pool(name="scp", bufs=3, space="PSUM"))
        avp = ctx.enter_context(tc.tile_pool(name="avp", bufs=2, space="PSUM"))

        BN = BPC * N  # 512

        # ---- inputs. DMA flows in issue order at full bandwidth after a
        # fixed ~9us engine-startup latency, so the k-interleaved qT/Wq
        # chunks (384KB per k-step) let the k-outer q-projection start ~1us
        # after first bytes instead of waiting for the full 3MB. kv[0]+Wv
        # next (vproj b0), then Wk/Wo/bo which are needed later.
        # Each dma_start costs the sync engine ~0.8us of serial trigger time,
        # so batch into k-PAIRS: 8 triggers for qT+Wq instead of 16, with the
        # k=0,1 chunks (375KB) arriving ~10us in for the k-outer qproj start.
        qt_sb = const.tile([128, 8 * BN], F16)
        qt_v = qt_sb[:].rearrange("p (k n) -> p k n", k=8)
        qt_dv = qT_d.rearrange("(k p) n -> p k n", p=128)
        wq_sb = const.tile([128, 8 * INNER], F16)
        wq_v = wq_sb[:].rearrange("p (k n) -> p k n", k=8)
        wq_dv = wq_d.rearrange("(k p) n -> p k n", p=128)
        # k=0 and k=1 as singles (first matmul waits only 375KB/2), rest as
        # pairs to bound the per-dma trigger tax.
        for ks, ke in ((0, 1), (1, 2), (2, 4), (4, 6), (6, 8)):
            nc.sync.dma_start(qt_v[:, ks:ke, :], qt_dv[:, ks:ke, :])
            nc.sync.dma_start(wq_v[:, ks:ke, :], wq_dv[:, ks:ke, :])

        kvt_tiles = {}

        def load_kvt(bb, split=False):
            t = kvt_pool.tile([128, 4 * M], F16, tag="kvt")
            tv = t[:].rearrange("p (k h m) -> p k h m", k=4, h=2)
            dv = kvT_d[bb].rearrange("(k p) (h m) -> p k h m", p=128, h=2)
            if split:
                return t, tv, dv
            nc.sync.dma_start(tv[:], dv[:])
            kvt_tiles[bb] = t

        # b0's kv and Wv stream as column/n2 halves interleaved so vproj's
        # first matmuls (t<8, n2=0) start one half-arrival earlier.
        kvt0, kvt0_v, kvt0_dv = load_kvt(0, split=True)
        kvt_tiles[0] = kvt0
        wv_sb = const.tile([128, 4 * INNER], F16)
        wv_v = wv_sb[:].rearrange("p (k h n) -> p k h n", k=4, h=2)
        wv_dv = wv_d.rearrange("(k p) (h n) -> p k h n", p=128, h=2)
        nc.sync.dma_start(kvt0_v[:, :, 0, :], kvt0_dv[:, :, 0, :])
        nc.sync.dma_start(wv_v[:, :, 0, :], wv_dv[:, :, 0, :])
        nc.sync.dma_start(kvt0_v[:, :, 1, :], kvt0_dv[:, :, 1, :])
        nc.sync.dma_start(wv_v[:, :, 1, :], wv_dv[:, :, 1, :])
        wk_sb = const.tile([128, 4 * INNER], F16)
        nc.sync.dma_start(
            wk_sb[:].rearrange("p (k n) -> p k n", k=4),
            wk_d.rearrange("(k p) n -> p k n", p=128),
        )
        wo_sb = const.tile([128, 8 * QD], F16)
        nc.sync.dma_start(
            wo_sb[:].rearrange("p (k n) -> p k n", k=8),
            wo_d.rearrange("(k p) n -> p k n", p=128),
        )
        bo_bc = const.tile([128, QD], F16)
        nc.sync.dma_start(bo_bc[:], bo_d[:])

        # ---- qhT projection: all 4 batches at once, chunk c = head pair.
        # Layout (c, b, hi, n): per (c, b) the two head-halves sit in adjacent
        # 128-col blocks, each with the complementary 64 partition rows zeroed,
        # so ONE K=128 N=256 scores matmul serves both heads of the pair.
        qh2 = const.tile([128, 8 * 2 * BN], F16)
        qh2v = qh2[:].rearrange("p (c b i n) -> p c b i n", c=8, b=BPC, i=2)
        nc.vector.memset(qh2v[64:128, :, :, 0, :], 0.0)
        nc.vector.memset(qh2v[0:64, :, :, 1, :], 0.0)
        # k-outer accumulation across all 8 PSUM banks (borrowed from the
        # pp/scp/avp rings, unused this early) so the first matmul only waits
        # for the k=0 qT/Wq chunks.
        qps = []
        for c in range(8):
            pool_c = pp if c < 3 else (scp if c < 6 else avp)
            qp = pool_c.tile([128, BN], F32, tag=("pp", "scp", "avp")[min(c // 3, 2)],
                             name=f"qp{c}")
            qps.append(qp)
        for k in range(8):
            for c in range(8):
                nc.tensor.matmul(
                    qps[c][:],
                    wq_sb[:, k * INNER + c * 128:k * INNER + (c + 1) * 128],
                    qt_sb[:, k * BN:(k + 1) * BN],
                    start=(k == 0), stop=(k == 7),
                )
        # evictions on Scalar (idle until the first EXP) so the Vector queue
        # is free for vproj's PSUM->vh casts right after.
        for c in range(8):
            nc.scalar.activation(
                qh2v[0:64, c, :, 0, :],
                qps[c][0:64, :].rearrange("p (b n) -> p b n", b=BPC),
                AF.Copy,
            )
            nc.scalar.activation(
                qh2v[64:128, c, :, 1, :],
                qps[c][64:128, :].rearrange("p (b n) -> p b n", b=BPC),
                AF.Copy,
            )

        vh_sb = vh_pool.tile([128, 16 * H * 65], F16, tag="vh")
        vh4 = vh_sb[:].rearrange("p (t h d) -> p t h d", h=H, d=65)
        nc.vector.memset(vh4[:, :, :, 64:65], 1.0)

        # Out-projection for a finished batch: fold odd heads into rows
        # 64-127 (one rectangular partition-shift DMA), K=128 accumulate
        # over the 8 inner chunks, add bias, store fp16 halves. Deferred
        # until after the NEXT batch's vh matmuls so its serialized tail
        # hides under PE work.
        def do_outproj(work):
            bb, o2 = work
            y_sb = y_pool.tile([128, QD], F16, tag="yp")
            yp0 = pp.tile([128, 512], F32, tag="pp")
            yp1 = pp.tile([128, 512], F32, tag="pp")
            yps = [yp0, yp1]
            def add_store(n2):
                nc.vector.tensor_add(
                    y_sb[:, n2 * 512:(n2 + 1) * 512],
                    yps[n2][:],
                    bo_bc[:, n2 * 512:(n2 + 1) * 512],
                )
                nc.sync.dma_start(y_d[bb, :, n2 * 512:(n2 + 1) * 512],
                                  y_sb[:, n2 * 512:(n2 + 1) * 512])

            # n2-outer: the n2=0 add+store hides under the n2=1 group's
            # eight matmuls instead of just the final one.
            for n2 in range(2):
                for c3 in range(8):
                    nc.tensor.matmul(
                        yps[n2][:],
                        o2[:, c3 * N:(c3 + 1) * N],
                        wo_sb[:, c3 * QD + n2 * 512:c3 * QD + (n2 + 1) * 512],
                        start=(c3 == 0), stop=(c3 == 7),
                    )
                add_store(n2)

        pending_proj = None
        for b in range(BPC):
            if b + 1 < BPC:
                load_kvt(b + 1)
            kvt_sb = kvt_tiles.pop(b)

            # ---- vh = kv @ Wv, kv tokens on partitions, 65-col blocks per
            # head. n2-outer so the first pass consumes only the first
            # Wv/kv column halves, matching the DMA arrival order for b0.
            for n2 in range(2):
                for t in range(16):
                    p = pp.tile([128, 512], F32, tag="pp")
                    for k in range(4):
                        nc.tensor.matmul(
                            p[:],
                            kvt_sb[:, k * M + t * 128:k * M + (t + 1) * 128],
                            wv_sb[:, k * INNER + n2 * 512:k * INNER + (n2 + 1) * 512],
                            start=(k == 0), stop=(k == 3),
                        )
                    nc.vector.tensor_copy(
                        vh4[:, t, n2 * 8:(n2 + 1) * 8, 0:64],
                        p[:].rearrange("p (h d) -> p h d", d=64),
                    )

            if pending_proj is not None:
                do_outproj(pending_proj)
                pending_proj = None

            # ---- per head pair: khT chunk -> scores -> exp; AV runs one
            # pair behind so the ACT exp latency hides under PE work.
            pending = None

            def do_av(avwork):
                at_list, cc = avwork
                p = avp.tile([128, 256], F32, tag="avp")
                for hi in range(2):
                    h = 2 * cc + hi
                    for t in range(16):
                        tg, j = t // 2, t % 2
                        nc.tensor.matmul(
                            p[0:65, hi * 128:(hi + 1) * 128],
                            vh4[:, t, h, :],
                            at_list[tg][:, j * 256 + hi * 128:j * 256 + (hi + 1) * 128],
                            start=(t == 0), stop=(t == 15),
                        )
                # normalize: denom (row 64) -> partition 0 (scalar ACT copy;
                # custom-DVE ops only work at partition base 0) -> 1/denom ->
                # broadcast down partitions -> multiply. No PE involvement.
                rb = rb_pool.tile([64, 2 * N], F32, tag="rb")
                dce = r_pool.tile([1, N], F32, tag="ce")
                dco = r_pool.tile([1, N], F32, tag="co")
                r32e = r_pool.tile([1, N], F32, tag="re")
                r32o = r_pool.tile([1, N], F32, tag="ro")
                nc.scalar.activation(dce[:], p[64:65, 0:128], AF.Copy)
                nc.scalar.activation(dco[:], p[64:65, 128:256], AF.Copy)
                nc.vector.reciprocal_approx_fast(r32e[:], dce[:])
                nc.vector.reciprocal_approx_fast(r32o[:], dco[:])
                nc.gpsimd.partition_broadcast(rb[:, 0:N], r32e[:])
                nc.gpsimd.partition_broadcast(rb[:, N:2 * N], r32o[:])
                # DVE output APs may sit at a different partition base than
                # the (base-aligned) inputs, so the odd head lands directly
                # in rows 64:128 -- no partition-fold DMA needed.
                nc.vector.tensor_mul(outt2[0:64, cc * N:(cc + 1) * N],
                                     p[0:64, 0:128], rb[:, 0:N])
                nc.vector.tensor_mul(outt2[64:128, cc * N:(cc + 1) * N],
                                     p[0:64, 128:256], rb[:, N:2 * N])

            outt2 = outt_pool.tile([128, 8 * N], F16, tag="outt2")
            for c in range(8):
                kht_sb = kht_pool.tile([128, M], F16, tag="kht")
                for n in range(4):
                    p = pp.tile([128, 512], F32, tag="pp")
                    for k in range(4):
                        nc.tensor.matmul(
                            p[:],
                            wk_sb[:, k * INNER + c * 128:k * INNER + (c + 1) * 128],
                            kvt_sb[:, k * M + n * 512:k * M + (n + 1) * 512],
                            start=(k == 0), stop=(k == 3),
                        )
                    nc.vector.tensor_copy(kht_sb[:, n * 512:(n + 1) * 512], p[:])

                at_tiles = []
                for tg in range(8):
                    sc = scp.tile([128, 512], F32, tag="scp")
                    for j in range(2):
                        t = tg * 2 + j
                        nc.tensor.matmul(
                            sc[:, j * 256:(j + 1) * 256],
                            kht_sb[:, t * 128:(t + 1) * 128],
                            qh2[:, (c * BPC + b) * 256:(c * BPC + b + 1) * 256],
                            start=True, stop=True,
                        )
                    at = attn_pool.tile([128, 512], F16, tag="attn")
                    nc.scalar.activation(at[:], sc[:], AF.Exp)
                    at_tiles.append(at)

                if pending is not None:
                    do_av(pending)
                pending = (at_tiles, c)
            do_av(pending)

            pending_proj = (b, outt2)
        do_outproj(pending_proj)

    nc.compile()
    return nc


def _get_nc():
    if "nc" not in _cached:
        _cached["nc"] = _build_nc()
    return _cached["nc"]


def kernel(q, kv, Wq, Wk, Wv, Wo, bo):
    from concourse.bass_utils import run_bass_kernel_spmd

    nc = _get_nc()

    wq16 = Wq.astype(np.float16)
    wk16 = Wk.astype(np.float16)
    wv16 = Wv.astype(np.float16)
    wo16 = Wo.astype(np.float16)
    bo16 = np.ascontiguousarray(
        np.broadcast_to(bo.reshape(1, QD), (128, QD)).astype(np.float16))

    scale = D ** -0.5  # 1/8, exact in fp16
    in_maps = []
    for i in range(NCORES):
        bs = slice(i * BPC, (i + 1) * BPC)
        # (BPC, N, QD) -> (QD, BPC, N) -> (QD, BPC*N), scale folded in
        qT = np.ascontiguousarray(
            np.transpose(q[bs] * scale, (2, 0, 1)).reshape(QD, BPC * N)
        ).astype(np.float16)
        kvT = np.ascontiguousarray(np.transpose(kv[bs], (0, 2, 1))).astype(
            np.float16
        )
        in_maps.append(
            {"qT": qT, "kvT": kvT, "Wq": wq16, "Wk": wk16, "Wv": wv16,
             "Wo": wo16, "bo": bo16}
        )

    _cached["in_maps"] = in_maps
    res = run_bass_kernel_spmd(nc, in_maps, list(range(NCORES)))
    out = np.concatenate([res.results[i]["y"] for i in range(NCORES)], axis=0)
    return out.astype(np.float32)
